# revision 24
# baseline (speedup 1.0000x reference)
"""DenseAtt kernel for Trainium2 (Bass/Tile), 8-core batch-parallel.

Math (per batch element b):
    s_left  = x @ W[:d]          # [n]
    s_right = x @ W[d:]          # [n]
    out[i,j] = sigmoid(s_left[i] + s_right[j] + bias) * adj[i,j]

Shapes: x [8, 2048, 128] f32, adj [8, 2048, 2048] f32, W [256] f32, b [] f32.
Sharding: one batch element per NeuronCore (B == n_cores == 8), no
collectives; full inputs in, full output out, gather on host.

Production path: _build_v2 (phase-separated). HW-measured on this pod
(8-core SPMD, per core): pure-read DMA streams at ~500 GB/s and
pure-write at ~600 GB/s, but mixed read+write traffic drops to
~365 GB/s aggregate (HBM bus turnaround), so interleaving load/store
per block (the old _build path, harness-measured 123.3 us) wastes ~2x.
_build_v2 instead:

  1. Streams the ENTIRE 16 MiB adj shard into one resident SBUF tile
     (128 KB/partition) as 8x 2 MiB DMAs on the in-order sync queue.
  2. Under that load stream: PE transposes x -> xT, computes s_left /
     s_right row, broadcasts s_right to all partitions via a PE ones
     outer-product; ACT runs the 16 per-block sigmoids (bias = s_left
     column, free); DVE multiplies att * adj IN PLACE over the resident
     adj slices.
  3. Streams all 16 MiB back out (8x 2 MiB stores) as a pure-write
     phase: stores sit behind all loads on the same in-order queue, and
     each store sem-waits on the DVE multiply of its blocks.

Gated-gap A/B timing (time_ab.py, chained reps isolated by an
ACT-sigmoid delay chain gated on kernel completion): ~200.9 us per
rep at gap=60 vs 191.7 us for the DMA phases alone => compute costs
~9 us on the critical path; isolated kernel estimate ~60-70 us vs the
123.3 us harness baseline. Variants measured worse: ld/st_bpi=4
(+5 us), bf16 att + DVE setup copies (+7 us), dual-queue DMA (no gain;
the two HWDGE queues do not overlap transfer phases).

repeat_full / gap are timing-only knobs; production is repeat_full=1,
gap=0 (no probe instructions emitted).
"""

from contextlib import ExitStack

import numpy as np

import concourse.bass as bass
import concourse.tile as tile
from concourse import bacc, mybir
from concourse.bass_utils import run_bass_kernel_spmd

N = 2048
D = 128
P = 128
NBLK = N // P  # 16
NCORES = 8

_cache = {}


def _build(
    adj_bufs=6,
    att_bufs=4,
    out_bufs=4,
    inplace_mult=False,
    blocks_per_iter=1,
    skip_setup=False,
    repeat=1,
    warm_act=False,
    gp_every=0,  # >0: every gp_every-th block's multiply runs on GPSIMD
    no_mult=False,  # timing-only: skip the multiply, store adj directly
    pe_setup=False,  # compute s_left/s_right via PE transposes+matmuls
    repeat_full=1,  # timing: chain the ENTIRE kernel (setup included) M times
) -> bass.Bass:
    assert repeat_full == 1 or (pe_setup and not skip_setup)
    f32 = mybir.dt.float32
    nc = bacc.Bacc("TRN2", target_bir_lowering=False, debug=False)
    hq = nc.sync  # legacy builder: all DMAs on the sync queue

    x = nc.dram_tensor("x", [N, D], f32, kind="ExternalInput").ap()
    adj = nc.dram_tensor("adj", [N, N], f32, kind="ExternalInput").ap()
    bb = nc.dram_tensor("bb", [P, 1], f32, kind="ExternalInput").ap()
    if pe_setup:
        wc = nc.dram_tensor("wc", [P, 2], f32, kind="ExternalInput").ap()
    else:
        wl = nc.dram_tensor("wl", [P, D], f32, kind="ExternalInput").ap()
        wr = nc.dram_tensor("wr", [P, D], f32, kind="ExternalInput").ap()
        sr_scr = nc.dram_tensor("sr_scr", [N], f32, kind="Internal").ap()
    out = nc.dram_tensor("out", [N, N], f32, kind="ExternalOutput").ap()

    H = NBLK // 2
    xd = x.rearrange("(i p) d -> p i d", p=P)

    with ExitStack() as ctx:
        tc = ctx.enter_context(tile.TileContext(nc))
        const = ctx.enter_context(tc.tile_pool(name="const", bufs=1))
        rot = ctx.enter_context(
            tc.tile_pool(name="rot", bufs=2 if repeat_full > 1 else 1)
        )
        adj_pool = ctx.enter_context(tc.tile_pool(name="adjp", bufs=adj_bufs))
        att_pool = ctx.enter_context(tc.tile_pool(name="attp", bufs=att_bufs))
        out_pool = (
            None
            if inplace_mult or no_mult
            else ctx.enter_context(tc.tile_pool(name="outp", bufs=out_bufs))
        )

        # --- true constants ---
        bb_t = const.tile([P, 1], f32)
        nc.sync.dma_start(bb_t[:], bb)
        if pe_setup:
            ident_h = nc.inline_tensor(np.eye(P, dtype=np.float32), name="ident")
            ident_t = const.tile([P, P], f32)
            nc.sync.dma_start(ident_t[:], ident_h.ap())
            wc_t = const.tile([P, 2], f32)
            nc.sync.dma_start(wc_t[:], wc)
            tp_pool = ctx.enter_context(tc.tile_pool(name="tp", bufs=2, space="PSUM"))
            slp_pool = ctx.enter_context(
                tc.tile_pool(name="slp", bufs=2, space="PSUM")
            )
            srp_pool = ctx.enter_context(
                tc.tile_pool(name="srp", bufs=4, space="PSUM")
            )
        else:
            wl_t = const.tile([P, D], f32)
            nc.sync.dma_start(wl_t[:], wl)
            wr_t = const.tile([P, D], f32)
            nc.sync.dma_start(wr_t[:], wr)

        if warm_act:
            # Load the sigmoid ACT table at t=0, off the critical path.
            warm = const.tile([P, 1], f32)
            nc.vector.memset(warm[:], 0.0)
            nc.scalar.activation(
                warm[:], warm[:], mybir.ActivationFunctionType.Sigmoid
            )

        for _rep in range(repeat_full):
            # x staged as two half tiles (separate deps -> earlier consumers):
            # x_th[h][p, i*D+d] = x[(h*H+i)*128+p, d]
            x_th = []
            for h in range(2):
                xh = rot.tile([P, H * D], f32, tag=f"x{h}")
                hq.dma_start(
                    xh[:].rearrange("p (i d) -> p i d", d=D),
                    xd[:, h * H : (h + 1) * H, :],
                )
                x_th.append(xh)

            def xblk(i):  # [128, 128] slice of x for row-block i (p, d)
                h, o = divmod(i, H)
                return x_th[h][:, o * D : (o + 1) * D]

            sl_t = rot.tile([P, NBLK], f32, tag="sl")  # s_left + b
            sr_b = rot.tile([P, N], f32, tag="srb")  # s_right bcast

            if skip_setup:
                # Profiling-only variant: fake s to isolate main-loop time.
                nc.vector.memset(sl_t[:], 0.0)
                nc.vector.memset(sr_b[:], 0.0)
            elif pe_setup:
                # PE computes everything: xT via identity-transposes, then
                # s_left = xT_i.T @ wc[:,0], s_right row = wc[:,1].T @ xT_i.
                # DVE does no setup work at all.
                xt_t = rot.tile([P, N], f32, tag="xt")  # xT: [d, (i n)]
                GRP = 4  # transposes per PSUM bank
                for g in range(NBLK // GRP):
                    tp = tp_pool.tile([P, GRP * P], f32)
                    for o in range(GRP):
                        i = g * GRP + o
                        nc.tensor.transpose(
                            tp[:, o * P : (o + 1) * P], xblk(i), ident_t[:]
                        )
                    nc.scalar.copy(
                        xt_t[:, g * GRP * P : (g + 1) * GRP * P], tp[:]
                    )

                # s_right row chunks: [1, 128] per block -> 4x [1, 512]
                sr_row = rot.tile([1, N], f32, tag="sr_row")
                for c in range(4):
                    src = srp_pool.tile([1, 4 * P], f32)
                    for o in range(4):
                        i = c * 4 + o
                        nc.tensor.matmul(
                            src[:, o * P : (o + 1) * P],
                            wc_t[:, 1:2],
                            xt_t[:, i * P : (i + 1) * P],
                        )
                    nc.scalar.copy(
                        sr_row[:, c * 4 * P : (c + 1) * 4 * P], src[:]
                    )
                nc.gpsimd.partition_broadcast(sr_b[:], sr_row[:])

                # s_left columns [128, 16], bias b folded in the ACT copy
                sl_ps = slp_pool.tile([P, NBLK], f32)
                for i in range(NBLK):
                    nc.tensor.matmul(
                        sl_ps[:, i : i + 1],
                        xt_t[:, i * P : (i + 1) * P],
                        wc_t[:, 0:1],
                    )
                nc.scalar.add(sl_t[:], sl_ps[:], bb_t[:, 0:1])
            else:
                # --- s_left / s_right columns [128, 16] via DVE ---
                sl_raw = rot.tile([P, NBLK], f32, tag="sl_raw")
                sr_t = rot.tile([P, NBLK], f32, tag="sr_cols")
                prod = rot.tile([P, N], f32, tag="prod")
                for w_t, s_t in ((wr_t, sr_t), (wl_t, sl_raw)):
                    for h in range(2):
                        wb = w_t[:].unsqueeze(1).broadcast_to([P, H, D])
                        p3 = prod[:, h * H * D : (h + 1) * H * D].rearrange(
                            "p (i d) -> p i d", d=D
                        )
                        nc.vector.tensor_tensor(
                            p3,
                            x_th[h][:].rearrange("p (i d) -> p i d", d=D),
                            wb,
                            op=mybir.AluOpType.mult,
                        )
                        nc.vector.reduce_sum(
                            s_t[:, h * H : (h + 1) * H],
                            p3,
                            axis=mybir.AxisListType.X,
                        )

                nc.vector.tensor_scalar_add(sl_t[:], sl_raw[:], bb_t[:, 0:1])

                # --- s_right -> row layout in DRAM, reload, broadcast ---
                nc.sync.dma_start(sr_scr.rearrange("(i p) -> p i", p=P), sr_t[:])
                sr_row = rot.tile([1, N], f32, tag="sr_row")
                nc.sync.dma_start(sr_row[:], sr_scr.unsqueeze(0))
                nc.gpsimd.partition_broadcast(sr_b[:], sr_row[:])

            # --- main loop over row blocks ---
            BPI = blocks_per_iter
            W_ = N * BPI
            for it_rep in range(repeat * (NBLK // BPI)):
                it = it_rep % (NBLK // BPI)
                i0 = it * BPI
                adj_t = adj_pool.tile([P, W_], f32)
                if BPI == 1:
                    nc.sync.dma_start(adj_t[:], adj[i0 * P : (i0 + 1) * P, :])
                else:
                    nc.sync.dma_start(
                        adj_t[:].rearrange("p (u j) -> p u j", u=BPI),
                        adj[i0 * P : (i0 + BPI) * P, :].rearrange(
                            "(u p) j -> p u j", p=P
                        ),
                    )
                att_t = att_pool.tile([P, W_], f32)
                for u in range(BPI):
                    nc.scalar.activation(
                        att_t[:, u * N : (u + 1) * N],
                        sr_b[:],
                        mybir.ActivationFunctionType.Sigmoid,
                        bias=sl_t[:, i0 + u : i0 + u + 1],
                        scale=1.0,
                    )
                if no_mult:
                    o_t = adj_t
                else:
                    o_t = adj_t if inplace_mult else out_pool.tile([P, W_], f32)
                    eng = (
                        nc.gpsimd
                        if gp_every and it_rep % gp_every == gp_every - 1
                        else nc.vector
                    )
                    eng.tensor_tensor(
                        o_t[:], att_t[:], adj_t[:], op=mybir.AluOpType.mult
                    )
                if BPI == 1:
                    nc.sync.dma_start(out[i0 * P : (i0 + 1) * P, :], o_t[:])
                else:
                    nc.sync.dma_start(
                        out[i0 * P : (i0 + BPI) * P, :].rearrange(
                            "(u p) j -> p u j", p=P
                        ),
                        o_t[:].rearrange("p (u j) -> p u j", u=BPI),
                    )

    nc.compile()
    return nc


def _emit_gap(nc, gap, gtile, gscr, out, adj, adj_res, x_th, st_bpi, no_stores):
    """Timing probe: gate an ACT-sigmoid delay chain on kernel completion,
    then write markers every next-rep load depends on. See time_ab.py."""
    f32 = mybir.dt.float32
    # Probe runs ENTIRELY on GPSIMD (kernel never uses it) so the delay
    # chain serializes with the DMA phases only through the gate/markers.
    if not no_stores:
        # RAW on every store: read a corner of each store's out range.
        nst = NBLK // st_bpi
        nc.sync.dma_start(
            gscr[0:nst, :],
            out.rearrange("(u q) j -> u q j", q=st_bpi * P)[
                :, st_bpi * P - 1, 0:16
            ],
        )
        nc.scalar.copy(gtile[0:nst, 0:16], gscr[0:nst, :])
    else:
        # No stores: gate on the last writers of adj_res (loads or DVE)
        # by reading one non-marker word from every slice.
        nc.scalar.copy(
            gtile[:, 16:32].rearrange("p (i j) -> p i j", j=1),
            adj_res[:].rearrange("p (i j) -> p i j", j=N)[:, :, 1:2],
        )
    for _ in range(gap):
        nc.scalar.activation(
            gtile[:], gtile[:], mybir.ActivationFunctionType.Sigmoid
        )
    # Markers: head word of every adj slice + of the x tiles, so ALL of the
    # next rep's loads wait for gap end.
    nc.scalar.copy(
        adj_res[:].rearrange("p (i j) -> p i j", j=N)[:, :, 0:1],
        gtile[:, 0:NBLK].rearrange("p (i j) -> p i j", j=1),
    )
    for h in range(2):
        nc.scalar.copy(x_th[h][:, 0:1], gtile[:, 0:1])


def _build_v2(
    ld_bpi=2,  # blocks per load-DMA instruction
    st_bpi=2,  # blocks per store-DMA instruction
    att_bufs=3,
    repeat_full=1,  # timing: chain the ENTIRE kernel M times
    act_splits=1,  # sigmoid instructions per block (ACT pipelining)
    no_compute=False,  # timing-only: skip setup+sigmoid+mult, store adj back
    no_stores=False,  # timing-only: skip the store phase
    gap=0,  # timing-only: gated ACT-sigmoid idle chain between chained reps
    att_bf16=False,  # att tiles in bf16: halves ACT-write/DVE-read SBUF traffic
    dve_copies=False,  # setup PSUM->SBUF copies on DVE instead of ACT
    head_scalar=False,  # issue const/x DMAs on the scalar HWDGE queue so the
    # sync queue starts the adj stream immediately
) -> bass.Bass:
    """Phase-separated DenseAtt kernel.

    HW-measured (8-core SPMD, per core): pure-read DMA streams at
    ~504 GB/s and pure-write at ~599 GB/s, but mixed read+write traffic
    drops to ~365 GB/s aggregate (HBM bus turnaround). So instead of
    interleaving load/compute/store per block, this kernel:

      phase 1: streams the ENTIRE 16 MiB adj shard into SBUF (~33 us),
               with setup (PE transposes, s_left/s_right) and the
               sigmoid+multiply pipeline running under it;
      phase 2: streams all 16 MiB of out back (~28 us), reading the
               multiply results written IN PLACE over the adj tile.

    All DMAs sit on the single in-order sync (SP) HWDGE queue: loads are
    emitted first, so no store can start before the last load issues, and
    each store waits (semaphore) on the DVE multiply of its blocks.
    """
    f32 = mybir.dt.float32
    nc = bacc.Bacc("TRN2", target_bir_lowering=False, debug=False)

    x = nc.dram_tensor("x", [N, D], f32, kind="ExternalInput").ap()
    adj = nc.dram_tensor("adj", [N, N], f32, kind="ExternalInput").ap()
    bb = nc.dram_tensor("bb", [P, 1], f32, kind="ExternalInput").ap()
    wc = nc.dram_tensor("wc", [P, 2], f32, kind="ExternalInput").ap()
    out = nc.dram_tensor("out", [N, N], f32, kind="ExternalOutput").ap()

    H = NBLK // 2
    xd = x.rearrange("(i p) d -> p i d", p=P)

    with ExitStack() as ctx:
        tc = ctx.enter_context(tile.TileContext(nc))
        const = ctx.enter_context(tc.tile_pool(name="const", bufs=1))
        rot = ctx.enter_context(tc.tile_pool(name="rot", bufs=1))
        big = ctx.enter_context(tc.tile_pool(name="big", bufs=1))
        att_pool = ctx.enter_context(tc.tile_pool(name="attp", bufs=att_bufs))
        tp_pool = ctx.enter_context(tc.tile_pool(name="tp", bufs=2, space="PSUM"))
        slp_pool = ctx.enter_context(tc.tile_pool(name="slp", bufs=2, space="PSUM"))
        srp_pool = ctx.enter_context(tc.tile_pool(name="srp", bufs=4, space="PSUM"))

        # --- true constants (tiny DMAs; optionally off the sync queue) ---
        hq = nc.scalar if head_scalar else nc.sync
        bb_t = const.tile([P, 1], f32)
        hq.dma_start(bb_t[:], bb)
        ident_h = nc.inline_tensor(np.eye(P, dtype=np.float32), name="ident")
        ident_t = const.tile([P, P], f32)
        hq.dma_start(ident_t[:], ident_h.ap())
        wc_t = const.tile([P, 2], f32)
        hq.dma_start(wc_t[:], wc)
        ones_t = const.tile([1, P], f32)
        nc.vector.memset(ones_t[:], 1.0)

        # Load the sigmoid ACT table at t=0, off the critical path.
        warm = const.tile([P, 1], f32)
        nc.vector.memset(warm[:], 0.0)
        nc.scalar.activation(warm[:], warm[:], mybir.ActivationFunctionType.Sigmoid)

        if gap:
            # Timing-probe scratch: gate tile + engine-clocked delay chain.
            gtile = const.tile([P, N], f32)
            nc.vector.memset(gtile[:], 0.25)
            gscr = const.tile([8, 16], f32)

        for _rep in range(repeat_full):
            # x staged as two half tiles (separate deps -> earlier consumers)
            x_th = []
            for h in range(2):
                xh = rot.tile([P, H * D], f32, tag=f"x{h}")
                hq.dma_start(
                    xh[:].rearrange("p (i d) -> p i d", d=D),
                    xd[:, h * H : (h + 1) * H, :],
                )
                x_th.append(xh)

            def xblk(i):  # [128, 128] slice of x for row-block i (p, d)
                h, o = divmod(i, H)
                return x_th[h][:, o * D : (o + 1) * D]

            # --- adj resident tile: the whole shard, loaded in ld_bpi blocks
            adj_res = big.tile([P, NBLK * N], f32, tag="adjres")

            def ablk(i, n=1):  # [128, n*2048] slice for row-blocks i..i+n
                return adj_res[:, i * N : (i + n) * N]

            for it in range(NBLK // ld_bpi):
                i0 = it * ld_bpi
                nc.sync.dma_start(
                    ablk(i0, ld_bpi).rearrange("p (u j) -> p u j", u=ld_bpi),
                    adj[i0 * P : (i0 + ld_bpi) * P, :].rearrange(
                        "(u p) j -> p u j", p=P
                    ),
                )

            if no_compute:
                if not no_stores:
                    for it in range(NBLK // st_bpi):
                        i0 = it * st_bpi
                        nc.sync.dma_start(
                            out[i0 * P : (i0 + st_bpi) * P, :].rearrange(
                                "(u p) j -> p u j", p=P
                            ),
                            ablk(i0, st_bpi).rearrange("p (u j) -> p u j", u=st_bpi),
                        )
                if gap:
                    _emit_gap(
                        nc, gap, gtile, gscr, out, adj, adj_res, x_th,
                        st_bpi, no_stores,
                    )
                continue

            # --- setup: PE transposes + matmuls -> s_left, s_right ---
            sl_t = rot.tile([P, NBLK], f32, tag="sl")  # s_left + b
            sr_b = rot.tile([P, N], f32, tag="srb")  # s_right bcast
            xt_t = rot.tile([P, N], f32, tag="xt")  # xT: [d, (i n)]
            GRP = 4  # transposes per PSUM bank
            for g in range(NBLK // GRP):
                tp = tp_pool.tile([P, GRP * P], f32, tag="tp")
                for o in range(GRP):
                    i = g * GRP + o
                    nc.tensor.transpose(
                        tp[:, o * P : (o + 1) * P], xblk(i), ident_t[:]
                    )
                (nc.vector.tensor_copy if dve_copies else nc.scalar.copy)(
                    xt_t[:, g * GRP * P : (g + 1) * GRP * P], tp[:]
                )

            # s_right row chunks: [1, 128] per block -> 4x [1, 512]
            sr_row = rot.tile([1, N], f32, tag="sr_row")
            for c in range(4):
                src = srp_pool.tile([1, 4 * P], f32)
                for o in range(4):
                    i = c * 4 + o
                    nc.tensor.matmul(
                        src[:, o * P : (o + 1) * P],
                        wc_t[:, 1:2],
                        xt_t[:, i * P : (i + 1) * P],
                    )
                nc.scalar.copy(sr_row[:, c * 4 * P : (c + 1) * 4 * P], src[:])
            # Broadcast sr_row to all 128 partitions via PE (ones outer
            # product) — keeps GPSIMD free for the timing probe.
            for c in range(4):
                bc = tp_pool.tile([P, 4 * P], f32, tag="tp")
                nc.tensor.matmul(
                    bc[:], ones_t[:], sr_row[:, c * 4 * P : (c + 1) * 4 * P]
                )
                (nc.vector.tensor_copy if dve_copies else nc.scalar.copy)(
                    sr_b[:, c * 4 * P : (c + 1) * 4 * P], bc[:]
                )

            # s_left columns [128, 16], bias b folded in the ACT copy
            sl_ps = slp_pool.tile([P, NBLK], f32)
            for i in range(NBLK):
                nc.tensor.matmul(
                    sl_ps[:, i : i + 1],
                    xt_t[:, i * P : (i + 1) * P],
                    wc_t[:, 0:1],
                )
            nc.scalar.add(sl_t[:], sl_ps[:], bb_t[:, 0:1])

            # --- compute pipeline: sigmoid (ACT) then in-place mult (DVE)
            att_dt = mybir.dt.bfloat16 if att_bf16 else f32
            CW = N // act_splits
            for i in range(NBLK):
                att_t = att_pool.tile([P, N], att_dt)
                for s in range(act_splits):
                    nc.scalar.activation(
                        att_t[:, s * CW : (s + 1) * CW],
                        sr_b[:, s * CW : (s + 1) * CW],
                        mybir.ActivationFunctionType.Sigmoid,
                        bias=sl_t[:, i : i + 1],
                        scale=1.0,
                    )
                nc.vector.tensor_tensor(
                    ablk(i), att_t[:], ablk(i), op=mybir.AluOpType.mult
                )

            # --- store phase: pure-write stream off the resident tile ---
            if not no_stores:
                for it in range(NBLK // st_bpi):
                    i0 = it * st_bpi
                    nc.sync.dma_start(
                        out[i0 * P : (i0 + st_bpi) * P, :].rearrange(
                            "(u p) j -> p u j", p=P
                        ),
                        ablk(i0, st_bpi).rearrange("p (u j) -> p u j", u=st_bpi),
                    )

            if gap:
                _emit_gap(
                    nc, gap, gtile, gscr, out, adj, adj_res, x_th,
                    st_bpi, no_stores,
                )

    nc.compile()
    return nc


# blocks_per_iter=2: 2MB DMAs (better HBM efficiency), DVE span 35.4us vs
# 36.6 at bpi=1, half the DMA/op fixed costs. HW-validated end-to-end:
# rel err 1.084e-5 (same as bpi=1).
PROD_CONFIG = dict(
    pe_setup=True,
    warm_act=True,
    blocks_per_iter=2,
    adj_bufs=4,
    att_bufs=2,
    out_bufs=3,
)


USE_V2 = True


def _get_nc() -> bass.Bass:
    if "nc" not in _cache:
        _cache["nc"] = _build_v2() if USE_V2 else _build(**PROD_CONFIG)
    return _cache["nc"]


def _declared_inputs(nc):
    import concourse.mybir as _mb

    names = set()
    for alloc in nc.m.functions[0].allocations:
        if isinstance(alloc, _mb.MemoryLocationSet) and alloc.kind == "ExternalInput":
            names.add(alloc.memorylocations[0].name)
    return names


def _in_maps(x, adj, W, b, nc=None):
    x = np.ascontiguousarray(np.asarray(x, dtype=np.float32))
    adj = np.ascontiguousarray(np.asarray(adj, dtype=np.float32))
    W = np.asarray(W, dtype=np.float32)
    b = np.float32(np.asarray(b, dtype=np.float32))
    avail = {
        "wl": lambda: np.ascontiguousarray(np.broadcast_to(W[:D], (P, D))),
        "wr": lambda: np.ascontiguousarray(np.broadcast_to(W[D:], (P, D))),
        "wc": lambda: np.ascontiguousarray(W.reshape(2, D).T),
        "bb": lambda: np.full((P, 1), b, dtype=np.float32),
    }
    if nc is None:
        nc = _get_nc()
    names = _declared_inputs(nc)
    shared = {k: f() for k, f in avail.items() if k in names}
    return [{"x": x[c], "adj": adj[c], **shared} for c in range(NCORES)]


def run(x, adj, W, b, trace=False):
    import os

    if not trace:
        # This axon client image has no NTFF profile hook
        # (antenv.axon_hooks); an inherited BASS_TRACE=1 would crash the
        # run on that import, so force tracing off.
        os.environ["BASS_NEVER_TRACE"] = "1"
    nc = _get_nc()
    res = run_bass_kernel_spmd(
        nc,
        _in_maps(x, adj, W, b, nc=nc),
        core_ids=list(range(NCORES)),
        trace=trace,
    )
    out = np.stack([res.results[c]["out"] for c in range(NCORES)], axis=0)
    return out, res


def kernel(x, adj, W, b):
    out, _ = run(x, adj, W, b)
    return out

